# revision 74
# baseline (speedup 1.0000x reference)
"""BERT self-attention Bass kernel for 8 Trainium2 NeuronCores.

Problem: hidden_states [2, 2048, 768], 12 heads x 64 dim, fp32.

Sharding (zero-communication): core c in 0..7 handles batch b = c//4 and
head-group g = c%4 (3 heads). Host pre-lays inputs per core (fp16):
  - hsT   [768, 2048]  hidden[b].T
  - w     [768, 640]   QKV weight columns packed into 5 psum row-groups:
                       g0=[q0|q1] g1=[k0|k1] g2=[q2|v2] g3=[k2|v1] g4=[v0]
                       (pairing keeps each head's Q and K partition-aligned;
                       softmax 1/8 folded into Wq)
  - bias  [128, 5]     combined bias per row-group (fp32)
  - maskr [128, 17]    attention_mask[b] column-tiled (col i = keys
                       i*128..i*128+127); col 16 unused
  - ident [128, 128]   identity (PE transposes + PE filler)

Device pipeline per core (fp16 matmuls, fp32 psum accumulate):
  1. Phase 1 = QKV groups g2 (q2|v2) and g3 (k2|v1) ONLY - exactly the 8
     sb-unit accumulators that fit in the 8 psum banks, and exactly what
     head 2's attention (plus v1) needs. kc-major rounds track the hsT
     chunk arrival order (small tensors DMA first: maskr gates em = the
     exp-table load and every later ACT op); warm-up/filler matmuls keep
     the HAM clock at 2.4 GHz through the chunk-arrival gaps. K lands in
     zero-padded ktp tiles so scores contract K=128 with a full-128 rhs
     (a K=64 partition-offset variant costs ~100ns on the first matmul
     after every weight-geometry switch).
  2. V transposed to [t, d] via PE transposes into psum scratch, then
     V_aug[t, 65] = exp(mask_t) * [V | 1] on DVE (denominator falls out of
     the PV matmul's ones column; exact for -inf masks too). vaug2 builds
     right after phase 1 (its scratch keeps the ps_b slot-rotation parity
     so ctx tiles never wait on long-lived predecessors); vaug0/vaug1
     build mid-attention with scratch in the all-short-lived ps_c
     rotation. (An XBAR-DMA transpose was tried and costs ~1.1us/tile ON
     the issuing engine.)
  3. Attention per (head, s-block of 512), heads 2,0,1, per t-pair step:
     2 scores matmuls -> one exp over [128,1024] on ACT (unnormalized, no
     max-sub: scores are O(6) by construction) -> 2 PV matmuls into
     ctxT[65, s]. PV emission trails TWO steps behind (sc(k+2) is emitted
     before PV(k)), so each exp's input is ready one full exp early: ACT
     runs its 96 exps back-to-back at ~1005ns (its roofline), which is
     the kernel's floor. The deferred work (g0, g1, g4 = 72 matmuls, V
     transposes) pulls into the PE's ~100ns/step slack against a
     deadline schedule; psum->sbuf drains ride ACT (pre-attention) or
     the DVE FIFO, ordered so K copies never block a scores step.
Host: divide rows 0..63 by row 64, transpose to [s, d], interleave heads.
"""

import os

import numpy as np

import concourse.mybir as mybir
import concourse.tile as tile
from concourse import bacc
from concourse.bass_utils import run_bass_kernel_spmd

F32 = mybir.dt.float32
F16 = mybir.dt.float16

B = 2
S = 2048
HID = 768
NH = 12          # total heads
D = 64           # head dim
NHL = 3          # heads per core
DG = NHL * D     # 192 cols of each W per core
QKV = 640        # packed QKVT row space (5 groups of 128)
KC = HID // 128  # 6 contraction chunks
NG = 5           # psum row-groups of QKVT
GW = [128, 128, 128, 128, 64]     # real rows per group
NT = S // 128    # 16 key tiles
SBW = 512        # s-block width
NSB = S // SBW   # 4 s-blocks

# (group, offset) per quantity and head. Head 2's Q, K, AND V live in
# g2/g3 so phase 1 (which only has psum banks for 8 sb-units = g2+g3)
# delivers everything head 2's attention needs; g4 (v0) + g0/g1 are
# computed inside the attention slack.
QPOS = [(0, 0), (0, 64), (2, 0)]
KPOS = [(1, 0), (1, 64), (3, 0)]
VPOS = [(4, 0), (3, 64), (2, 64)]
HORDER = [2, 0, 1]

LAST_EXEC_TIME_NS = None

_CACHED_NC = None


def _build_nc():
    nc = bacc.Bacc("TRN2", target_bir_lowering=False, debug=False, num_devices=8)

    hsT_d = nc.dram_tensor("hsT", [HID, S], F16, kind="ExternalInput")
    w_d = nc.dram_tensor("w", [HID, QKV], F16, kind="ExternalInput")
    bias_d = nc.dram_tensor("bias", [128, NG], F32, kind="ExternalInput")
    maskr_d = nc.dram_tensor("maskr", [128, NT + 1], F32, kind="ExternalInput")
    ident_d = nc.dram_tensor("ident", [128, 128], F16, kind="ExternalInput")
    out_d = nc.dram_tensor("ctxa", [NHL, D + 1, S], F32, kind="ExternalOutput")

    with tile.TileContext(nc) as tc:
        with (
            tc.tile_pool(name="const", bufs=1) as cp,
            tc.tile_pool(name="qkvt", bufs=1) as qp,
            tc.tile_pool(name="vaug", bufs=1) as vp,
            tc.tile_pool(name="probs", bufs=4) as pp,
            tc.tile_pool(name="oc", bufs=3) as op,
            tc.tile_pool(name="ps_a", bufs=2, space="PSUM") as ps_a,
            tc.tile_pool(name="ps_b", bufs=2, space="PSUM") as ps_b,
            tc.tile_pool(name="ps_c", bufs=2, space="PSUM") as ps_c,
        ):
            # --- constants / inputs ---
            # hs chunks stream on the SP HWDGE ring; weights and small
            # tensors on the ACT ring so descriptor generation for both
            # proceeds in parallel and the first QKV matmul starts early.
            ident = cp.tile([128, 128], F16, tag="ident")
            nc.scalar.dma_start(ident[:], ident_d.ap())
            # small tensors FIRST on the ACT ring: maskr gates em (and with
            # it the exp table load + every later ACT op), so it must not
            # queue behind the w chunks
            bias_sb = cp.tile([128, NG], F32, tag="bias")
            nc.scalar.dma_start(bias_sb[:], bias_d.ap())
            maskr_sb = cp.tile([128, NT + 1], F32, tag="maskr")
            nc.scalar.dma_start(maskr_sb[:], maskr_d.ap())
            w_sb = cp.tile([128, KC, QKV], F16, tag="w")
            w_ap = w_d.ap().rearrange("(kc p) n -> p kc n", p=128)
            # warm-filler source first in the gpsimd queue: the PE warm-up
            # must not wait behind SWDGE descriptor generation
            wfsrc = cp.tile([128, 16], F16, tag="wfsrc")
            nc.gpsimd.memset(wfsrc[:], 1.0)
            hs = []
            rings = [nc.sync, nc.gpsimd, nc.scalar]
            ri = 0
            for kc in range(KC):
                t = cp.tile([128, S], F16, tag=f"hsT{kc}", name=f"hsT{kc}")
                # each chunk split across two rings so it completes sooner
                # after its stream starts (chunk arrival paces phase 1)
                for half in range(2):
                    rings[ri % 3].dma_start(
                        t[half * 64 : (half + 1) * 64, :],
                        hsT_d.ap()[
                            kc * 128 + half * 64 : kc * 128 + (half + 1) * 64,
                            :,
                        ],
                    )
                    ri += 1
                nc.scalar.dma_start(
                    w_sb[:, kc : kc + 1, :], w_ap[:, kc : kc + 1, :]
                )
                hs.append(t)

            # em[t] = exp(mask_t), folded into V_aug below
            em = cp.tile([128, NT], F32, tag="em")
            nc.scalar.activation(
                em[:], maskr_sb[:, :NT], mybir.ActivationFunctionType.Exp
            )

            # --- QKVT = w.T @ hsT (d-major) ---
            qkvt = [
                qp.tile([128, S], F16, tag=f"qkvt{g}", name=f"qkvt{g}")
                for g in range(NG)
            ]
            # K weights per head in [128, S] tiles with the other 64
            # partitions zeroed: scores matmuls contract over K=128 with a
            # full-128-partition rhs. (Contracting K=64 via partition-offset
            # slices was tried and costs ~100ns on the first matmul after
            # every weight-geometry switch - the zero-padded form keeps the
            # PE at a uniform 128-row geometry all through attention.)
            ktp = [
                qp.tile([128, S], F16, tag=f"ktp{h}", name=f"ktp{h}")
                for h in range(NHL)
            ]
            nc.gpsimd.memset(ktp[0][64:128, :], 0.0)
            nc.gpsimd.memset(ktp[1][0:64, :], 0.0)
            nc.gpsimd.memset(ktp[2][64:128, :], 0.0)

            # Only the two copies sc(0) needs go on ACT - everything else
            # queues on DVE so exp(0) isn't stuck behind Identity copies.
            ACT_COPIES = {(2, 0), (3, 0)}

            def qkv_group(gi, pool, tag, sb_lo, sb_n, nsb=1):
                """Per-matmul closures for group gi over s-blocks
                [sb_lo, sb_lo+sb_n). Returns (units, accs): units[a][kc][i]
                with nsb s-blocks per accumulator; the kc==KC-1 closure
                drains the accumulated s-block to qkvt[gi] with bias."""
                gw = GW[gi]
                units, accs = [], []
                for sb0 in range(sb_lo, sb_lo + sb_n, nsb):
                    # acc allocated lazily at the kc==0 closure so pool slot
                    # rotation follows pull order, not group-creation order
                    box = [None]
                    accs.append(box)
                    by_kc = []
                    for kc in range(KC):
                        row = []
                        for i in range(nsb):
                            s0 = (sb0 + i) * SBW

                            def mm(kc=kc, i=i, box=box, s0=s0, gw=gw, gi=gi,
                                   nsb=nsb):
                                if box[0] is None:
                                    box[0] = pool.tile(
                                        [128, nsb * SBW], F32, tag=tag,
                                        name="acc",
                                    )
                                acc = box[0]
                                nc.tensor.matmul(
                                    acc[:gw, i * SBW : (i + 1) * SBW],
                                    w_sb[:, kc, gi * 128 : gi * 128 + gw],
                                    hs[kc][:, s0 : s0 + SBW],
                                    start=(kc == 0),
                                    stop=(kc == KC - 1),
                                )
                                if kc == KC - 1:
                                    sl = slice(s0, s0 + SBW)
                                    use_act = (gi, s0 // SBW) in ACT_COPIES

                                    def cp(dst, rows, b0, act=None):
                                        nr = rows.stop - rows.start
                                        a = acc[
                                            rows.start : rows.stop,
                                            i * SBW : (i + 1) * SBW,
                                        ]
                                        bias_ap = bias_sb[
                                            b0 : b0 + nr, gi : gi + 1
                                        ]
                                        if use_act if act is None else act:
                                            nc.scalar.activation(
                                                dst[rows, sl],
                                                a,
                                                mybir.ActivationFunctionType.Identity,
                                                bias=bias_ap,
                                            )
                                        else:
                                            nc.vector.tensor_scalar_add(
                                                dst[rows, sl], a, bias_ap
                                            )

                                    lo, hi = slice(0, 64), slice(64, 128)
                                    if gi == 1:  # k0|k1 -> ktp0/ktp1
                                        cp(ktp[0], lo, 0)
                                        cp(ktp[1], hi, 64)
                                    elif gi == 3:  # k2 -> ktp2, v1 stays
                                        cp(ktp[2], lo, 0)
                                        cp(qkvt[3], hi, 64, act=False)
                                    else:
                                        cp(qkvt[gi], slice(0, gw), 0)

                            row.append(mm)
                        by_kc.append(row)
                    units.append(by_kc)
                return units, accs

            # V_aug built with DMA XBAR transposes ([64,128] sbuf -> [128,64]
            # sbuf per key-tile, on the otherwise-idle SP/ACT rings) and a
            # gpsimd in-place em-multiply: no PE transposes, no psum
            # scratch, so the ctx psum rotation stays hazard-free.
            vaug = [
                vp.tile([128, NT, D + 1], F16, tag=f"vaug{h}", name=f"vaug{h}")
                for h in range(NHL)
            ]

            def make_vaug_part(h, t0, nt, pool, tag):
                """PE-transpose V key-tiles [t0, t0+nt) into [t, d] and
                em-multiply on DVE. tp scratch: ps_b for vaug2 (rotates
                against fast-draining predecessors), ps_c for the
                mid-attention vaug0/1 (all-short-lived rotation). An
                XBAR-DMA transpose was tried instead and costs ~1.1us PER
                TILE on the issuing engine - far worse than the PE."""
                ti, off = VPOS[h]
                vt = qkvt[ti]
                vg = vaug[h]
                tp = pool.tile([128, nt * D], F16, tag=tag, name="tp")
                for j in range(nt):
                    tt = t0 + j
                    nc.tensor.transpose(
                        tp[:, j * D : (j + 1) * D],
                        vt[off : off + D, tt * 128 : (tt + 1) * 128],
                        ident[off : off + D, off : off + D],
                    )
                nc.vector.tensor_tensor(
                    vg[:, t0 : t0 + nt, :D],
                    tp[:].rearrange("p (j d) -> p j d", d=D),
                    em[:, t0 : t0 + nt]
                    .rearrange("p (j o) -> p j o", o=1)
                    .broadcast_to([128, nt, D]),
                    mybir.AluOpType.mult,
                )
                if t0 == 0:
                    nc.gpsimd.tensor_copy(
                        vg[:, :, D : D + 1],
                        em[:, :NT].rearrange("p (j o) -> p j o", o=1),
                    )

            # PE warm-up with no DMA dependency: tiny matmuls on the
            # memset tile keep the PE busy from engine-init onward so the
            # HAM clock gate is warming before the first hsT chunk lands.
            # Pre-round fillers own psum tiles (allocated BEFORE the phase-1
            # accumulators so the slot rotation completes instantly); the
            # per-round fillers instead accumulate an all-zero product into
            # a live g2 accumulator corner, since every psum slot is held by
            # a phase-1 accumulator until chunk 5 drains.
            wfz = cp.tile([128, 16], F16, tag="wfz")
            nc.gpsimd.memset(wfz[:], 0.0)
            wf_tiles = iter(
                [
                    ps_c.tile([128, SBW], F32, tag="acc", name=f"wf{i}")
                    for i in range(10)
                ]
            )

            def warm(n):
                for _ in range(n):
                    wf = next(wf_tiles, None)
                    if wf is None:
                        return
                    nc.tensor.matmul(
                        wf[:16, :16],
                        wfsrc[:],
                        wfsrc[:],
                        start=True,
                        stop=True,
                        skip_group_check=True,
                    )

            warm(6)

            # Just-in-time warm fillers: weights sourced from the first
            # half-chunk of hsT, so they fire as chunk 0's top half lands
            # and the PE clock is ramping right before round 0 begins
            # (pure pre-fillers run out ~9us before the first chunk).
            for j in range(4):
                wf = next(wf_tiles, None)
                if wf is not None:
                    nc.tensor.matmul(
                        wf[:16, :16],
                        hs[0][0:64, j * 16 : (j + 1) * 16],
                        wfsrc[0:64, :],
                        start=True,
                        stop=True,
                        skip_group_check=True,
                    )

            # Phase 1 = g2 (q2|v2) and g3 (k2|v1) only: with pending-2 PV
            # emission, head 2's attention needs exactly these 8 sb-units,
            # and 8 psum banks is all there is. kc-major rounds track the
            # hsT chunk arrival order; everything else (g4=v0, g0, g1, all
            # V transposes) runs inside the attention slack.
            g2u, g2accs = qkv_group(2, ps_a, "ps", 0, 4, nsb=2)
            g3bu, _ = qkv_group(3, ps_b, "ps", 0, 2, nsb=1)
            g3cu, _ = qkv_group(3, ps_c, "acc", 2, 2, nsb=1)

            def zwarm(n):
                for _ in range(n):
                    nc.tensor.matmul(
                        g2accs[0][0][:16, :16],
                        wfsrc[:],
                        wfz[:],
                        start=False,
                        stop=False,
                        skip_group_check=True,
                    )

            # Round order: g3b0 (k2 sb0, whose ACT copy gates the first
            # scores matmul) drains first at kc5, then g2 (whose copies
            # gate the vaug2 transposes), then the rest of g3.
            # early rounds are DMA-paced and need fillers to keep the PE
            # clock up; late rounds are PE-backlogged and need none.
            # The kc5 matmuls of everything sc(0) does NOT need (g3b1,
            # g2a1, g3c*) defer into the first pull steps, so the PE
            # reaches sc(0) five matmuls (and their copy chain) sooner.
            NWARM = [0, 2, 2, 2, 0, 0]
            for kc in range(KC):
                if kc:
                    zwarm(NWARM[kc])
                g3bu[0][kc][0]()
                g2u[0][kc][0]()
                g2u[0][kc][1]()
                if kc < KC - 1:
                    g3bu[1][kc][0]()
                    g2u[1][kc][0]()
                    g2u[1][kc][1]()
                    g3cu[0][kc][0]()
                    g3cu[1][kc][0]()
            kc5_deferred = [
                g3bu[1][KC - 1][0],
                g2u[1][KC - 1][0],
                g2u[1][KC - 1][1],
                g3cu[0][KC - 1][0],
                g3cu[1][KC - 1][0],
            ]
            # vaug2 quarter 0 (tiles 0-3, needs only q2-sb0's ACT copy) is
            # the only pre-attention V work; tiles 4-15 pull at step 2
            # paired with a dummy slot that keeps the ps_b rotation parity
            # (each ctx tile waits only a short-lived predecessor)
            make_vaug_part(2, 0, 4, ps_b, "ps")

            # --- attention ---
            # Deferred QKV work + V transposes run inside the attention
            # slack, pulled a few matmuls per step against a deadline
            # schedule (forced = cumulative items needed by end of step s).
            def vaug_unit(h, t0, nt, pool, tag):
                def unit(h=h, t0=t0, nt=nt, pool=pool, tag=tag):
                    make_vaug_part(h, t0, nt, pool, tag)

                return unit

            g1u, _ = qkv_group(1, ps_c, "acc", 0, 4, nsb=1)
            g0u, _ = qkv_group(0, ps_c, "acc", 0, 4, nsb=1)
            g4u, _ = qkv_group(4, ps_c, "acc", 0, 4, nsb=1)

            def flat(units):
                return [units[a][kc][0] for a in range(len(units))
                        for kc in range(KC)]

            g1f, g0f, g4f = flat(g1u), flat(g0u), flat(g4u)
            # h1's needs first (q1 = g0, v1 = g4 + its vaug halves), then
            # the rest of g0, then h0's k0/v0 (g1) late - matching the
            # relaxed deadlines that HORDER [2,1,0] buys.
            def v2rest_unit():
                make_vaug_part(2, 4, 12, ps_b, "ps")
                dm = ps_b.tile([128, 4], F32, tag="ps", name="dm")
                nc.vector.tensor_copy(dm[:, 0:1], em[:, 0:1])

            # k0/k1 (g1) ahead of the v0 work so the urgent ktp copies sit
            # at the front of the DVE FIFO; vaug halves right after the g4
            # units feeding them (their tp scratch is in ps_c, whose slots
            # are all short-lived - no rotation caps needed).
            interleave = (
                kc5_deferred + [v2rest_unit]          # items 0-4, 5
                + g1f[0:6] + g0f[0:6] + g4f[0:6] + g4f[6:12] + g1f[6:12]
                + [vaug_unit(0, 0, 8, ps_c, "acc")]   # item 36
                + g1f[12:18] + g1f[18:24] + g4f[12:18] + g4f[18:24]
                + [vaug_unit(0, 8, 8, ps_c, "acc")]   # item 61
                + g0f[6:12] + g0f[12:18] + g0f[18:24]
                + [vaug_unit(1, 0, 8, ps_c, "acc"),
                   vaug_unit(1, 8, 8, ps_c, "acc")]   # items 80, 81
            )
            NITEMS = len(interleave)
            forced = [(0, 2), (1, 5), (3, 6), (27, 18), (29, 30),
                      (31, 37), (33, 43), (34, 49), (35, 55), (36, 61),
                      (37, 62), (38, 68), (46, 74), (54, 80), (62, 81),
                      (66, 82)]
            NSTEPS_TOT = NHL * NSB * (NT // 2)
            cum_target = []
            prev = 0
            for s_ in range(NSTEPS_TOT):
                t_ = max(prev, (NITEMS * (s_ + 1) + 61) // 62)
                for fs, fn in forced:
                    if s_ >= fs:
                        t_ = max(t_, fn)
                t_ = min(t_, NITEMS)
                cum_target.append(t_)
                prev = t_
            ipos = 0
            gstep = 0
            NSTEP = NT // 2

            # software-pipelined emission: the PV pair of step k is emitted
            # AFTER sc(k+2), two steps behind. The scores pair of step k+2
            # only waits on its psum buffer (freed when exp(k) completes),
            # so ACT's exp chain runs back-to-back; PV(k) then starts with
            # exp(k)'s probs long since written, so its weight load overlaps
            # the preceding matmul instead of stalling on the probs sem.
            pending = []  # (h, ctx, st, pr, oc_args) awaiting PV emission

            def flush_one():
                if not pending:
                    return
                (ph, pctx, pst, ppr, poc) = pending.pop(0)
                for half in range(2):
                    tt = pst * 2 + half
                    nc.tensor.matmul(
                        pctx[: D + 1, :],
                        vaug[ph][:, tt, :],
                        ppr[:, half * SBW : (half + 1) * SBW],
                        start=(tt == 0),
                        stop=(tt == NT - 1),
                    )
                if poc is not None:
                    h_, s0_ = poc
                    oc = op.tile([128, SBW], F32, tag="oc", name="oc")
                    nc.vector.tensor_copy(oc[: D + 1, :], pctx[: D + 1, :])
                    nc.sync.dma_start(
                        out_d.ap()[h_, :, s0_ : s0_ + SBW],
                        oc[: D + 1, :],
                    )

            QTILE = [0, 0, 2]  # rhs tile per head (full 128 partitions)
            for h in HORDER:
                qt, kt = qkvt[QTILE[h]], ktp[h]
                for sbk in range(NSB):
                    s0 = sbk * SBW
                    ctx = ps_b.tile([128, SBW], F32, tag="ps", name="ctx")
                    for st in range(NSTEP):  # t-pair steps
                        sc = ps_a.tile([128, 2 * SBW], F32, tag="ps", name="sc")
                        for half in range(2):
                            tt = st * 2 + half
                            nc.tensor.matmul(
                                sc[:, half * SBW : (half + 1) * SBW],
                                kt[:, tt * 128 : (tt + 1) * 128],
                                qt[:, s0 : s0 + SBW],
                                start=True,
                                stop=True,
                            )
                        while ipos < cum_target[gstep]:
                            interleave[ipos]()
                            ipos += 1
                        pr = pp.tile([128, 2 * SBW], F16, tag="pr", name="pr")
                        nc.scalar.activation(
                            pr[:], sc[:], mybir.ActivationFunctionType.Exp
                        )
                        pending.append(
                            (h, ctx, st, pr,
                             (h, s0) if st == NSTEP - 1 else None)
                        )
                        if len(pending) > 2:
                            flush_one()
                        gstep += 1
            while pending:
                flush_one()

    nc.compile()
    return nc


def _get_nc():
    global _CACHED_NC
    if _CACHED_NC is None:
        _CACHED_NC = _build_nc()
    return _CACHED_NC


def kernel(
    hidden_states, attention_mask, Wq, bq, Wk, bk, Wv, bv
) -> np.ndarray:
    global LAST_EXEC_TIME_NS
    hidden_states = np.asarray(hidden_states, dtype=np.float32)
    attention_mask = np.asarray(attention_mask, dtype=np.float32)
    Wq = np.asarray(Wq, dtype=np.float32)
    Wk = np.asarray(Wk, dtype=np.float32)
    Wv = np.asarray(Wv, dtype=np.float32)
    bq = np.asarray(bq, dtype=np.float32)
    bk = np.asarray(bk, dtype=np.float32)
    bv = np.asarray(bv, dtype=np.float32)

    scale = 1.0 / np.sqrt(np.float32(D))

    in_maps = []
    for c in range(8):
        b, g = divmod(c, 4)
        cols = slice(g * DG, (g + 1) * DG)
        wq = Wq[:, cols] * scale
        wk = Wk[:, cols]
        wv = Wv[:, cols]
        w = np.zeros((HID, QKV), dtype=np.float32)
        bcat = np.zeros(QKV, dtype=np.float32)
        bq_, bk_, bv_ = bq[cols] * scale, bk[cols], bv[cols]
        for h in range(NHL):
            for (pos, mat, bb) in (
                (QPOS[h], wq, bq_),
                (KPOS[h], wk, bk_),
                (VPOS[h], wv, bv_),
            ):
                gi, off = pos
                r0 = gi * 128 + off
                w[:, r0 : r0 + D] = mat[:, h * D : (h + 1) * D]
                bcat[r0 : r0 + D] = bb[h * D : (h + 1) * D]
        bias = np.ascontiguousarray(bcat.reshape(NG, 128).T)
        maskr = np.zeros((128, NT + 1), dtype=np.float32)
        maskr[:, :NT] = attention_mask[b, 0, 0, :].reshape(NT, 128).T
        in_maps.append(
            {
                "hsT": np.ascontiguousarray(hidden_states[b].T).astype(np.float16),
                "w": w.astype(np.float16),
                "bias": bias,
                "maskr": maskr,
                "ident": np.eye(128, dtype=np.float16),
            }
        )

    nc = _get_nc()
    trace = bool(os.environ.get("BASS_KERNEL_TRACE"))
    res = run_bass_kernel_spmd(nc, in_maps, list(range(8)), trace=trace)
    LAST_EXEC_TIME_NS = res.exec_time_ns

    out = np.empty((B, S, HID), dtype=np.float32)
    for c in range(8):
        b, g = divmod(c, 4)
        ctxa = res.results[c]["ctxa"]  # [3, 65, 2048]
        for hl in range(NHL):
            ctx = ctxa[hl, :D, :] / ctxa[hl, D : D + 1, :]  # [64, 2048]
            out[b, :, g * DG + hl * D : g * DG + (hl + 1) * D] = ctx.T
    return out

